# revision 31
# baseline (speedup 1.0000x reference)
"""Trainium2 Bass kernel for nn_DiffNet (2-layer LSTM encoder/decoder + FC head).

Sharding: tensor-parallel over the hidden/gate dimension across 8 NeuronCores.
Core k owns hidden rows [k*128, (k+1)*128) of both LSTM layers (and the
matching rows of each of the 4 gates); the FC head is replicated (cheaper than
putting a third collective on the decode critical path).  Activations are
[hidden_partitions, batch] so the batch (256) is the matmul moving dimension.

Per-step hidden-state exchange is one 64KB AllGather per layer per step,
scheduled so each collective's ~7us mesh time hides under the *other* layer's
matmuls.  Gate pre-activations are packed two-per-PSUM-bank ([128, 2, B] f32 =
one 2KB bank) so a full LSTM step holds 4 banks instead of 8, leaving room
for the decode FC to keep all 8 of its M-tiles accumulating concurrently.
Matmul k-loops are ordered k-outer so they chase the 4-chunk gather DMAs
(split across the SP and Activation HWDGE queues where queue order allows).

Self-contained: hardcodes all shapes; host-side numpy only reshapes/slices.
"""

import numpy as np

L = 2
H = 1024
XD = 192
YD = 64
IN = XD + YD  # 256
B = 256
PRE_LEN = 64
FWD_LEN = 48
NCORES = 8
SL = H // NCORES  # 128 hidden rows per core
KT_H = H // 128  # 8 K-tiles to contract over a full hidden vector
NGATE = 4

# gate packing: bank A holds (g, i), bank B holds (f, o) — torch gate order
# in the weights is (i, f, g, o) = indices (0, 1, 2, 3).
PACK = [(0, 0, 2), (0, 1, 0), (1, 0, 1), (1, 1, 3)]  # (bank, slot, gate_idx)

_CACHE = {}


def _shard_host(inputs):
    """Build per-core input dicts (numpy only: slice / transpose / reshape)."""
    f32 = np.float32

    pre_x = np.asarray(inputs["pre_x"], f32)
    pre_y = np.asarray(inputs["pre_y"], f32)
    fwd_x = np.asarray(inputs["forward_x"], f32)

    # Encoder input, step-major, [t, p(128), kt(2), b] so the DMA is contiguous
    xy = np.concatenate([pre_x, pre_y], axis=2)  # (PRE, B, IN)
    xpre = (
        xy.transpose(0, 2, 1)  # (PRE, IN, B)
        .reshape(PRE_LEN, 2, 128, B)
        .transpose(0, 2, 1, 3)  # (PRE, 128, 2, B)
        .astype(np.float16)
    )
    # Decoder exogenous input: [t, in(192), b]
    xfwd = fwd_x.transpose(0, 2, 1).astype(np.float16)  # (FWD, 192, B)

    w_ih_0 = np.asarray(inputs["w_ih_0"], f32).reshape(NGATE, H, IN)
    w_hh_0 = np.asarray(inputs["w_hh_0"], f32).reshape(NGATE, H, H)
    w_ih_1 = np.asarray(inputs["w_ih_1"], f32).reshape(NGATE, H, H)
    w_hh_1 = np.asarray(inputs["w_hh_1"], f32).reshape(NGATE, H, H)
    b0 = (np.asarray(inputs["b_ih_0"], f32) + np.asarray(inputs["b_hh_0"], f32)).reshape(NGATE, H)
    b1 = (np.asarray(inputs["b_ih_1"], f32) + np.asarray(inputs["b_hh_1"], f32)).reshape(NGATE, H)
    fc_w1 = np.asarray(inputs["fc_w1"], f32)
    fc_b1 = np.asarray(inputs["fc_b1"], f32)
    fc_w2 = np.asarray(inputs["fc_w2"], f32)
    fc_b2 = np.asarray(inputs["fc_b2"], f32)

    def lhsT_hid(w, k):
        """(4, H, K) gate-major weight -> lhsT [128, KT, 4, 128] for core k."""
        sl = w[:, k * SL : (k + 1) * SL, :]  # (4, 128, K)
        kdim = sl.shape[2]
        kt = kdim // 128
        return (
            sl.transpose(2, 0, 1)  # (K, 4, 128)
            .reshape(kt, 128, NGATE, SL)
            .transpose(1, 0, 2, 3)  # (128, kt, 4, 128)
            .reshape(128, kt * NGATE * SL)
            .copy()
        )

    maps = []
    for k in range(NCORES):
        sl = slice(k * SL, (k + 1) * SL)
        w0xT = lhsT_hid(w_ih_0, k)  # (128, 2*4*128)
        west = w_ih_0[:, sl, XD:]  # (4, 128, 64)
        westT = west.transpose(2, 0, 1).reshape(YD, NGATE * SL).copy()  # (64, 512)
        whh0T = lhsT_hid(w_hh_0, k)  # (128, 8*4*128)
        wih1T = lhsT_hid(w_ih_1, k)
        whh1T = lhsT_hid(w_hh_1, k)
        fcw1T = (
            fc_w1.T.reshape(KT_H, 128, H).transpose(1, 0, 2).reshape(128, KT_H * H).copy()
        )
        fcw2T = (
            fc_w2.T.reshape(KT_H, 128, YD).transpose(1, 0, 2).reshape(128, KT_H * YD).copy()
        )
        m = {
            "xpre": xpre,
            "xfwd": xfwd,
            "w0xT": w0xT.astype(np.float16),
            "westT": westT.astype(np.float16),
            "whh0T": whh0T.astype(np.float16),
            "wih1T": wih1T.astype(np.float16),
            "whh1T": whh1T.astype(np.float16),
            "fcw1T": fcw1T.astype(np.float16),
            "fcw2T": fcw2T.astype(np.float16),
            "b0": b0[:, sl].T.copy(),  # (128, 4)
            "b1": b1[:, sl].T.copy(),
            "fcb1": fc_b1.reshape(KT_H, 128).T.copy(),  # (128, 8): bias per M-tile
            "fcb2r": fc_b2.reshape(1, YD).astype(np.float16).copy(),
            "fcb2": fc_b2.reshape(YD, 1).copy(),
            "onesr": np.ones((1, B), np.float16),
            "lastyT": pre_y[-1].T.copy(),  # (64, 256)
        }
        maps.append(m)
    return maps


def _build_program():
    import concourse.mybir as mybir
    import concourse.tile as tile
    from concourse import bacc

    dt = mybir.dt
    AF = mybir.ActivationFunctionType
    F32 = dt.float32
    FR = dt.float16  # matmul operand dtype: FWL stays on, ~8x bf16 precision

    nc = bacc.Bacc("TRN2", target_bir_lowering=False, debug=False, num_devices=NCORES)

    t_xpre = nc.dram_tensor("xpre", [PRE_LEN, 128, 2, B], FR, kind="ExternalInput")
    t_xfwd = nc.dram_tensor("xfwd", [FWD_LEN, XD, B], FR, kind="ExternalInput")
    t_w0xT = nc.dram_tensor("w0xT", [128, 2 * NGATE * SL], FR, kind="ExternalInput")
    t_westT = nc.dram_tensor("westT", [YD, NGATE * SL], FR, kind="ExternalInput")
    t_whh0T = nc.dram_tensor("whh0T", [128, KT_H * NGATE * SL], FR, kind="ExternalInput")
    t_wih1T = nc.dram_tensor("wih1T", [128, KT_H * NGATE * SL], FR, kind="ExternalInput")
    t_whh1T = nc.dram_tensor("whh1T", [128, KT_H * NGATE * SL], FR, kind="ExternalInput")
    t_fcw1T = nc.dram_tensor("fcw1T", [128, KT_H * H], FR, kind="ExternalInput")
    t_fcw2T = nc.dram_tensor("fcw2T", [128, KT_H * YD], FR, kind="ExternalInput")
    t_b0 = nc.dram_tensor("b0", [128, NGATE], F32, kind="ExternalInput")
    t_b1 = nc.dram_tensor("b1", [128, NGATE], F32, kind="ExternalInput")
    t_fcb1 = nc.dram_tensor("fcb1", [128, KT_H], F32, kind="ExternalInput")
    t_fcb2r = nc.dram_tensor("fcb2r", [1, YD], FR, kind="ExternalInput")
    t_fcb2 = nc.dram_tensor("fcb2", [YD, 1], F32, kind="ExternalInput")
    t_onesr = nc.dram_tensor("onesr", [1, B], FR, kind="ExternalInput")
    t_lastyT = nc.dram_tensor("lastyT", [YD, B], F32, kind="ExternalInput")
    t_out = nc.dram_tensor("est_out", [FWD_LEN, YD, B], F32, kind="ExternalOutput")

    RG = [list(range(NCORES))]

    with tile.TileContext(nc) as tc:
        with (
            tc.tile_pool(name="const", bufs=1) as const,
            tc.tile_pool(name="xload", bufs=4) as xload,
            tc.tile_pool(name="state", bufs=4) as state,
            tc.tile_pool(name="gact", bufs=6) as gact,
            tc.tile_pool(name="hfull", bufs=4) as hfull,
            tc.tile_pool(name="psum", bufs=8, space="PSUM") as psum,
            tc.tile_pool(name="dbounce", bufs=8, space="DRAM") as dbounce,
            tc.tile_pool(name="dshared", bufs=8, space="DRAM") as dshared,
        ):
            # ---- load constants ----
            w0xT = const.tile([128, 2, NGATE, SL], FR)
            nc.sync.dma_start(out=w0xT, in_=t_w0xT.ap().rearrange("p (k g m) -> p k g m", k=2, g=NGATE))
            westT = const.tile([YD, NGATE, SL], FR)
            nc.sync.dma_start(out=westT, in_=t_westT.ap().rearrange("p (g m) -> p g m", g=NGATE))
            whh0T = const.tile([128, KT_H, NGATE, SL], FR)
            nc.sync.dma_start(out=whh0T, in_=t_whh0T.ap().rearrange("p (k g m) -> p k g m", k=KT_H, g=NGATE))
            wih1T = const.tile([128, KT_H, NGATE, SL], FR)
            nc.sync.dma_start(out=wih1T, in_=t_wih1T.ap().rearrange("p (k g m) -> p k g m", k=KT_H, g=NGATE))
            whh1T = const.tile([128, KT_H, NGATE, SL], FR)
            nc.sync.dma_start(out=whh1T, in_=t_whh1T.ap().rearrange("p (k g m) -> p k g m", k=KT_H, g=NGATE))
            fcw1T = const.tile([128, KT_H, H], FR)
            nc.sync.dma_start(out=fcw1T, in_=t_fcw1T.ap().rearrange("p (k m) -> p k m", k=KT_H))
            fcw2T = const.tile([128, KT_H, YD], FR)
            nc.sync.dma_start(out=fcw2T, in_=t_fcw2T.ap().rearrange("p (k m) -> p k m", k=KT_H))
            b0 = const.tile([128, NGATE], F32)
            nc.sync.dma_start(out=b0, in_=t_b0.ap())
            b1 = const.tile([128, NGATE], F32)
            nc.sync.dma_start(out=b1, in_=t_b1.ap())
            fcb1 = const.tile([128, KT_H], F32)
            nc.sync.dma_start(out=fcb1, in_=t_fcb1.ap())
            fcb2r = const.tile([1, YD], FR)
            nc.sync.dma_start(out=fcb2r, in_=t_fcb2r.ap())
            fcb2c = const.tile([YD, 1], F32)
            nc.sync.dma_start(out=fcb2c, in_=t_fcb2.ap())
            onesr = const.tile([1, B], FR)
            nc.sync.dma_start(out=onesr, in_=t_onesr.ap())

            # ---- persistent state ----
            est = const.tile([YD, B], F32)  # replicated running estimate
            nc.sync.dma_start(out=est, in_=t_lastyT.ap())
            c0 = const.tile([128, B], F32)
            nc.vector.memset(c0, 0.0)
            c1 = const.tile([128, B], F32)
            nc.vector.memset(c1, 0.0)

            def zpair():
                """Two PSUM banks, 2 gates each: A=(g,i), B=(f,o)."""
                zA = psum.tile([128, 2, B], F32, tag="z", name="zA")
                zB = psum.tile([128, 2, B], F32, tag="z", name="zB")
                return zA, zB

            def gates_mm(z, w, k, h, start, stop):
                """One k-tile of all 4 gates into the packed banks.

                start=True clears the whole bank's has_written bits, so emit
                it only on the first MM into each bank (slot 0); the slot-1
                opener relies on cleared has_written = overwrite semantics.
                One stop per bank likewise (on the last MM, slot 1)."""
                for bank, slot, g in PACK:
                    nc.tensor.matmul(z[bank][:, slot, :], w[:, k, g, :], h[:, k, :],
                                     start=start and slot == 0,
                                     stop=stop and slot == 1)

            def ag_launch(hk, tag):
                """Bounce the local [128,B] fp16 slice to DRAM, trigger AG.
                The bounce rides the Activation HWDGE queue (right after the
                halfstep ACTs that produced hk) so the Sync queue holds only
                gather triggers and fires them the moment a mesh completes."""
                inb = dbounce.tile([128, B], FR, tag="agi" + tag, name="agi" + tag)
                nc.scalar.dma_start(out=inb, in_=hk[:])
                outb = dshared.tile([NCORES * 128, B], FR, tag="ago" + tag,
                                    name="ago" + tag, addr_space="Shared")
                nc.gpsimd.collective_compute(
                    "AllGather", mybir.AluOpType.bypass, replica_groups=RG,
                    ins=[inb[:].opt()], outs=[outb[:].opt()],
                )
                return outb

            def ag_gather(outb, tag, plan):
                """DMA the gathered [8*128,B] DRAM buffer into SBUF
                [128, KT_H, B].  plan: list of (ktile_lo, ktile_hi, engine)
                chunks so k-outer consumers can chase the arrivals."""
                full = hfull.tile([128, KT_H, B], FR, tag="hf" + tag, name="hf" + tag)
                src = outb[:].rearrange("(k p) b -> p k b", p=128)
                for lo, hi, eng in plan:
                    eng.dma_start(out=full[:, lo:hi, :], in_=src[:, lo:hi, :])
                return full

            def lstm_halfstep(z, bias, cprev, tagp):
                """Gate activations + cell update from packed banks (zA, zB).
                Returns (c_new, h_new); h_new fp16 for the next matmuls."""
                zA, zB = z
                gg = gact.tile([128, B], F32, tag="gg", name="gg")
                gi = gact.tile([128, B], F32, tag="gi", name="gi")
                gf = gact.tile([128, B], F32, tag="gf", name="gf")
                go = gact.tile([128, B], F32, tag="go", name="go")
                nc.scalar.activation(gg, zA[:, 0, :], AF.Tanh, bias=bias[:, 2:3])
                nc.scalar.activation(gi, zA[:, 1, :], AF.Sigmoid, bias=bias[:, 0:1])
                nc.scalar.activation(gf, zB[:, 0, :], AF.Sigmoid, bias=bias[:, 1:2])
                nc.scalar.activation(go, zB[:, 1, :], AF.Sigmoid, bias=bias[:, 3:4])
                ig = gact.tile([128, B], F32, tag="ig", name="ig")
                nc.vector.tensor_mul(ig, gi, gg)
                fc_ = gact.tile([128, B], F32, tag="fc_", name="fc_")
                nc.vector.tensor_mul(fc_, gf, cprev)
                cnew = state.tile([128, B], F32, tag=tagp, name="cnew")
                nc.vector.tensor_add(cnew, fc_, ig)
                tc_ = gact.tile([128, B], F32, tag="tc_", name="tc_")
                nc.scalar.activation(tc_, cnew, AF.Tanh)
                hnew = state.tile([128, B], FR, tag=tagp + "h", name="hnew")
                nc.vector.tensor_mul(hnew, go, tc_)
                return cnew, hnew

            def load_xpre(s):
                # encode: gathers own DQ1 (Sync), so xt rides the Activation
                # queue where DQ10 is idle at body top
                xt = xload.tile([128, 2, B], FR, tag="x", name="xt")
                nc.scalar.dma_start(out=xt, in_=t_xpre.ap()[s])
                return xt

            def load_xfwd(t):
                # decode: the AG bounce-outs own DQ10 (Activation) mid-body,
                # so xt rides Sync/DQ1 which is idle there
                xt = xload.tile([128, 2, B], FR, tag="x", name="xt")
                nc.sync.dma_start(out=xt[:, 0, :], in_=t_xfwd.ap()[t, 0:128, :])
                nc.sync.dma_start(out=xt[0:64, 1, :], in_=t_xfwd.ap()[t, 128:XD, :])
                return xt

            # ================= encode =================
            h0f = None  # gathered h0_{s-1}
            h1f = None  # gathered h1_{s-2}
            xt = load_xpre(0)
            for s in range(PRE_LEN):
                # z0_s x-part — fills the AG(h0_{s-1}) window
                z0 = zpair()
                for kx in range(2):
                    gates_mm(z0, w0xT, kx, xt, start=(kx == 0), stop=(s == 0 and kx == 1))
                xt = load_xpre(s + 1) if s + 1 < PRE_LEN else None
                # z1_{s-1} whh1-part — window work too
                if s >= 1:
                    z1 = zpair()
                    if s >= 2:
                        for k in range(KT_H):
                            gates_mm(z1, whh1T, k, h1f, start=(k == 0), stop=False)
                # z0_s close (dep: h0f_{s-1}), k-outer chasing the gather
                if s >= 1:
                    for k in range(KT_H):
                        gates_mm(z0, whh0T, k, h0f, start=False, stop=(k == KT_H - 1))
                c0, h0k = lstm_halfstep(z0, b0, c0, "c0")
                ob0 = ag_launch(h0k, "0")
                # z1_{s-1} close — runs during AG(h0_s)
                if s >= 1:
                    for k in range(KT_H):
                        gates_mm(z1, wih1T, k, h0f, start=(s == 1 and k == 0),
                                 stop=(k == KT_H - 1))
                    c1, h1k = lstm_halfstep(z1, b1, c1, "c1")
                    ob1 = ag_launch(h1k, "1")
                # gathers on the Sync queue (a Scalar-queue chunk here would
                # stall the next halfstep's ACTs on an AG they don't need)
                h0f = ag_gather(ob0, "0", [(0, 4, nc.sync), (4, KT_H, nc.sync)])
                if s >= 1:
                    h1f = ag_gather(ob1, "1", [(0, 4, nc.sync), (4, KT_H, nc.sync)])

            # ---- encode flush: L1 of the last encode step (h1_63 = "top") ----
            z1 = zpair()
            for k in range(KT_H):
                gates_mm(z1, whh1T, k, h1f, start=(k == 0), stop=False)
            for k in range(KT_H):
                gates_mm(z1, wih1T, k, h0f, start=False, stop=(k == KT_H - 1))
            c1, h1k = lstm_halfstep(z1, b1, c1, "c1")
            ob1 = ag_launch(h1k, "1")
            h1f = ag_gather(ob1, "1", [(0, 4, nc.sync), (4, KT_H, nc.scalar)])

            # ================= decode =================
            xt = load_xfwd(0)
            for t in range(FWD_LEN):
                last = t == FWD_LEN - 1
                # z0_t partials (x + whh0@h0f_{t-1}) — run during AG(h1_{t-1})
                if not last:
                    z0 = zpair()
                    for bank, slot, g in PACK:
                        nc.tensor.matmul(z0[bank][:, slot, :], w0xT[:, 0, g, :],
                                         xt[:, 0, :], start=(slot == 0), stop=False)
                        nc.tensor.matmul(z0[bank][:, slot, :], w0xT[0:64, 1, g, :],
                                         xt[0:64, 1, :], start=False, stop=False)
                    for k in range(KT_H):
                        gates_mm(z0, whh0T, k, h0f, start=False, stop=False)
                    xt = load_xfwd(t + 1) if t + 1 < FWD_LEN - 1 else None

                # ---- FC head (replicated): est_t — dep h1f_{t-1} ----
                # k-outer: all 8 M-tiles accumulate concurrently in 4 packed
                # banks, so the matmuls chase the h1f gather chunks.
                ups = [psum.tile([128, 2, B], F32, tag="z", name=f"up{mb}")
                       for mb in range(KT_H // 2)]
                for k in range(KT_H):
                    for m in range(KT_H):
                        nc.tensor.matmul(ups[m // 2][:, m % 2, :],
                                         fcw1T[:, k, m * 128:(m + 1) * 128],
                                         h1f[:, k, :],
                                         start=(k == 0 and m % 2 == 0),
                                         stop=(k == KT_H - 1 and m % 2 == 1))
                pp = psum.tile([YD, B], F32, tag="z", name="pp")
                us = []
                for m in range(KT_H):
                    u = gact.tile([128, B], FR, tag="u", name="u")
                    nc.scalar.activation(u, ups[m // 2][:, m % 2, :], AF.Tanh,
                                         bias=fcb1[:, m:m + 1])
                    us.append(u)
                    if m >= 1:
                        nc.tensor.matmul(pp, fcw2T[:, m - 1, :], us[m - 1],
                                         start=(m == 1), stop=False)
                nc.tensor.matmul(pp, fcw2T[:, KT_H - 1, :], us[KT_H - 1],
                                 start=False, stop=False)
                # fc2 bias folded in as a K=1 outer product: pp += fcb2 x ones
                nc.tensor.matmul(pp, fcb2r[:, :], onesr[:, :], start=False, stop=True)
                estn = state.tile([YD, B], F32, tag="est", name="estn")
                nc.vector.tensor_add(estn, est, pp)
                nc.sync.dma_start(out=t_out.ap()[t], in_=estn)
                if last:
                    break
                est_r = state.tile([YD, B], FR, tag="estr", name="est_r")
                nc.vector.tensor_add(est_r, est, pp)  # same sum, fp16 out
                est = estn
                # close z0_t with the est K-tile
                for bank, slot, g in PACK:
                    nc.tensor.matmul(z0[bank][:, slot, :], westT[:, g, :], est_r,
                                     start=False, stop=(slot == 1))
                c0, h0k = lstm_halfstep(z0, b0, c0, "c0")
                ob0 = ag_launch(h0k, "0")
                # z1_t whh1-part — runs during AG(h0_t)
                z1 = zpair()
                for k in range(KT_H):
                    gates_mm(z1, whh1T, k, h1f, start=(k == 0), stop=False)
                h0f = ag_gather(ob0, "0", [(0, 2, nc.sync), (2, 4, nc.scalar),
                                           (4, 6, nc.sync), (6, 8, nc.scalar)])
                # z1_t close — dep h0f_t, k-outer chasing the gather
                for k in range(KT_H):
                    gates_mm(z1, wih1T, k, h0f, start=False, stop=(k == KT_H - 1))
                c1, h1k = lstm_halfstep(z1, b1, c1, "c1")
                ob1 = ag_launch(h1k, "1")
                h1f = ag_gather(ob1, "1", [(0, 4, nc.sync), (4, 8, nc.scalar)])

    nc.compile()
    return nc


def kernel(**inputs) -> np.ndarray:
    from concourse.bass_utils import run_bass_kernel_spmd

    key = "prog"
    if key not in _CACHE:
        _CACHE[key] = _build_program()
    nc = _CACHE[key]

    in_maps = _shard_host(inputs)
    res = run_bass_kernel_spmd(nc, in_maps, core_ids=list(range(NCORES)))
    est = np.asarray(res.results[0]["est_out"])  # (FWD, YD, B)
    return est.transpose(0, 2, 1).astype(np.float32).copy()  # (FWD, B, YD)


# revision 33
# speedup vs baseline: 1.2706x; 1.2706x over previous
"""Trainium2 Bass kernel for nn_DiffNet (2-layer LSTM encoder/decoder + FC head).

Sharding: tensor-parallel over the hidden/gate dimension across 8 NeuronCores.
Core k owns hidden rows [k*128, (k+1)*128) of both LSTM layers (and the
matching rows of each of the 4 gates); the FC head is replicated (cheaper than
putting a third collective on the decode critical path).  Activations are
[hidden_partitions, batch] so the batch (256) is the matmul moving dimension.

Per-step hidden-state exchange is one 64KB AllGather per layer per step,
scheduled so each collective's ~7us mesh time hides under the *other* layer's
matmuls.  Gate pre-activations are packed two-per-PSUM-bank ([128, 2, B] f32 =
one 2KB bank) so a full LSTM step holds 4 banks instead of 8, leaving room
for the decode FC to keep all 8 of its M-tiles accumulating concurrently.
Matmul k-loops are ordered k-outer so they chase the 4-chunk gather DMAs
(split across the SP and Activation HWDGE queues where queue order allows).

Self-contained: hardcodes all shapes; host-side numpy only reshapes/slices.
"""

import numpy as np

L = 2
H = 1024
XD = 192
YD = 64
IN = XD + YD  # 256
B = 256
PRE_LEN = 64
FWD_LEN = 48
NCORES = 8
SL = H // NCORES  # 128 hidden rows per core
KT_H = H // 128  # 8 K-tiles to contract over a full hidden vector
NGATE = 4

# gate packing: bank A holds (g, i), bank B holds (f, o) — torch gate order
# in the weights is (i, f, g, o) = indices (0, 1, 2, 3).
PACK = [(0, 0, 2), (0, 1, 0), (1, 0, 1), (1, 1, 3)]  # (bank, slot, gate_idx)

_CACHE = {}


def _shard_host(inputs):
    """Build per-core input dicts (numpy only: slice / transpose / reshape)."""
    f32 = np.float32

    pre_x = np.asarray(inputs["pre_x"], f32)
    pre_y = np.asarray(inputs["pre_y"], f32)
    fwd_x = np.asarray(inputs["forward_x"], f32)

    # Encoder input, step-major, [t, p(128), kt(2), b] so the DMA is contiguous
    xy = np.concatenate([pre_x, pre_y], axis=2)  # (PRE, B, IN)
    xpre = (
        xy.transpose(0, 2, 1)  # (PRE, IN, B)
        .reshape(PRE_LEN, 2, 128, B)
        .transpose(0, 2, 1, 3)  # (PRE, 128, 2, B)
        .astype(np.float16)
    )
    # Decoder exogenous input: [t, in(192), b]
    xfwd = fwd_x.transpose(0, 2, 1).astype(np.float16)  # (FWD, 192, B)

    w_ih_0 = np.asarray(inputs["w_ih_0"], f32).reshape(NGATE, H, IN)
    w_hh_0 = np.asarray(inputs["w_hh_0"], f32).reshape(NGATE, H, H)
    w_ih_1 = np.asarray(inputs["w_ih_1"], f32).reshape(NGATE, H, H)
    w_hh_1 = np.asarray(inputs["w_hh_1"], f32).reshape(NGATE, H, H)
    b0 = (np.asarray(inputs["b_ih_0"], f32) + np.asarray(inputs["b_hh_0"], f32)).reshape(NGATE, H)
    b1 = (np.asarray(inputs["b_ih_1"], f32) + np.asarray(inputs["b_hh_1"], f32)).reshape(NGATE, H)
    fc_w1 = np.asarray(inputs["fc_w1"], f32)
    fc_b1 = np.asarray(inputs["fc_b1"], f32)
    fc_w2 = np.asarray(inputs["fc_w2"], f32)
    fc_b2 = np.asarray(inputs["fc_b2"], f32)

    def lhsT_hid(w, k):
        """(4, H, K) gate-major weight -> lhsT [128, KT, 4, 128] for core k."""
        sl = w[:, k * SL : (k + 1) * SL, :]  # (4, 128, K)
        kdim = sl.shape[2]
        kt = kdim // 128
        return (
            sl.transpose(2, 0, 1)  # (K, 4, 128)
            .reshape(kt, 128, NGATE, SL)
            .transpose(1, 0, 2, 3)  # (128, kt, 4, 128)
            .reshape(128, kt * NGATE * SL)
            .copy()
        )

    maps = []
    for k in range(NCORES):
        sl = slice(k * SL, (k + 1) * SL)
        w0xT = lhsT_hid(w_ih_0, k)  # (128, 2*4*128)
        west = w_ih_0[:, sl, XD:]  # (4, 128, 64)
        westT = west.transpose(2, 0, 1).reshape(YD, NGATE * SL).copy()  # (64, 512)
        whh0T = lhsT_hid(w_hh_0, k)  # (128, 8*4*128)
        wih1T = lhsT_hid(w_ih_1, k)
        whh1T = lhsT_hid(w_hh_1, k)
        fcw1T = (
            fc_w1.T.reshape(KT_H, 128, H).transpose(1, 0, 2).reshape(128, KT_H * H).copy()
        )
        fcw2T = (
            fc_w2.T.reshape(KT_H, 128, YD).transpose(1, 0, 2).reshape(128, KT_H * YD).copy()
        )
        m = {
            "xpre": xpre,
            "xfwd": xfwd,
            "w0xT": w0xT.astype(np.float16),
            "westT": westT.astype(np.float16),
            "whh0T": whh0T.astype(np.float16),
            "wih1T": wih1T.astype(np.float16),
            "whh1T": whh1T.astype(np.float16),
            "fcw1T": fcw1T.astype(np.float16),
            "fcw2T": fcw2T.astype(np.float16),
            "b0": b0[:, sl].T.copy(),  # (128, 4)
            "b1": b1[:, sl].T.copy(),
            "fcb1": fc_b1.reshape(KT_H, 128).T.copy(),  # (128, 8): bias per M-tile
            "fcb2r": fc_b2.reshape(1, YD).astype(np.float16).copy(),
            "fcb2": fc_b2.reshape(YD, 1).copy(),
            "onesr": np.ones((1, B), np.float16),
            "lastyT": pre_y[-1].T.copy(),  # (64, 256)
        }
        maps.append(m)
    return maps


def _build_program():
    import concourse.mybir as mybir
    import concourse.tile as tile
    from concourse import bacc

    dt = mybir.dt
    AF = mybir.ActivationFunctionType
    F32 = dt.float32
    FR = dt.float16  # matmul operand dtype: FWL stays on, ~8x bf16 precision

    nc = bacc.Bacc("TRN2", target_bir_lowering=False, debug=False, num_devices=NCORES)

    t_xpre = nc.dram_tensor("xpre", [PRE_LEN, 128, 2, B], FR, kind="ExternalInput")
    t_xfwd = nc.dram_tensor("xfwd", [FWD_LEN, XD, B], FR, kind="ExternalInput")
    t_w0xT = nc.dram_tensor("w0xT", [128, 2 * NGATE * SL], FR, kind="ExternalInput")
    t_westT = nc.dram_tensor("westT", [YD, NGATE * SL], FR, kind="ExternalInput")
    t_whh0T = nc.dram_tensor("whh0T", [128, KT_H * NGATE * SL], FR, kind="ExternalInput")
    t_wih1T = nc.dram_tensor("wih1T", [128, KT_H * NGATE * SL], FR, kind="ExternalInput")
    t_whh1T = nc.dram_tensor("whh1T", [128, KT_H * NGATE * SL], FR, kind="ExternalInput")
    t_fcw1T = nc.dram_tensor("fcw1T", [128, KT_H * H], FR, kind="ExternalInput")
    t_fcw2T = nc.dram_tensor("fcw2T", [128, KT_H * YD], FR, kind="ExternalInput")
    t_b0 = nc.dram_tensor("b0", [128, NGATE], F32, kind="ExternalInput")
    t_b1 = nc.dram_tensor("b1", [128, NGATE], F32, kind="ExternalInput")
    t_fcb1 = nc.dram_tensor("fcb1", [128, KT_H], F32, kind="ExternalInput")
    t_fcb2r = nc.dram_tensor("fcb2r", [1, YD], FR, kind="ExternalInput")
    t_fcb2 = nc.dram_tensor("fcb2", [YD, 1], F32, kind="ExternalInput")
    t_onesr = nc.dram_tensor("onesr", [1, B], FR, kind="ExternalInput")
    t_lastyT = nc.dram_tensor("lastyT", [YD, B], F32, kind="ExternalInput")
    t_out = nc.dram_tensor("est_out", [FWD_LEN, YD, B], F32, kind="ExternalOutput")

    RG = [list(range(NCORES))]

    with tile.TileContext(nc) as tc:
        with (
            tc.tile_pool(name="const", bufs=1) as const,
            tc.tile_pool(name="xload", bufs=4) as xload,
            tc.tile_pool(name="state", bufs=4) as state,
            tc.tile_pool(name="gact", bufs=6) as gact,
            tc.tile_pool(name="hfull", bufs=4) as hfull,
            tc.tile_pool(name="psum", bufs=8, space="PSUM") as psum,
            tc.tile_pool(name="dbounce", bufs=8, space="DRAM") as dbounce,
            tc.tile_pool(name="dshared", bufs=8, space="DRAM") as dshared,
        ):
            # ---- load constants ----
            w0xT = const.tile([128, 2, NGATE, SL], FR)
            nc.sync.dma_start(out=w0xT, in_=t_w0xT.ap().rearrange("p (k g m) -> p k g m", k=2, g=NGATE))
            westT = const.tile([YD, NGATE, SL], FR)
            nc.sync.dma_start(out=westT, in_=t_westT.ap().rearrange("p (g m) -> p g m", g=NGATE))
            whh0T = const.tile([128, KT_H, NGATE, SL], FR)
            nc.sync.dma_start(out=whh0T, in_=t_whh0T.ap().rearrange("p (k g m) -> p k g m", k=KT_H, g=NGATE))
            wih1T = const.tile([128, KT_H, NGATE, SL], FR)
            nc.sync.dma_start(out=wih1T, in_=t_wih1T.ap().rearrange("p (k g m) -> p k g m", k=KT_H, g=NGATE))
            whh1T = const.tile([128, KT_H, NGATE, SL], FR)
            nc.sync.dma_start(out=whh1T, in_=t_whh1T.ap().rearrange("p (k g m) -> p k g m", k=KT_H, g=NGATE))
            fcw1T = const.tile([128, KT_H, H], FR)
            nc.sync.dma_start(out=fcw1T, in_=t_fcw1T.ap().rearrange("p (k m) -> p k m", k=KT_H))
            fcw2T = const.tile([128, KT_H, YD], FR)
            nc.sync.dma_start(out=fcw2T, in_=t_fcw2T.ap().rearrange("p (k m) -> p k m", k=KT_H))
            b0 = const.tile([128, NGATE], F32)
            nc.sync.dma_start(out=b0, in_=t_b0.ap())
            b1 = const.tile([128, NGATE], F32)
            nc.sync.dma_start(out=b1, in_=t_b1.ap())
            fcb1 = const.tile([128, KT_H], F32)
            nc.sync.dma_start(out=fcb1, in_=t_fcb1.ap())
            fcb2r = const.tile([1, YD], FR)
            nc.sync.dma_start(out=fcb2r, in_=t_fcb2r.ap())
            fcb2c = const.tile([YD, 1], F32)
            nc.sync.dma_start(out=fcb2c, in_=t_fcb2.ap())
            onesr = const.tile([1, B], FR)
            nc.sync.dma_start(out=onesr, in_=t_onesr.ap())

            # ---- persistent state ----
            est = const.tile([YD, B], F32)  # replicated running estimate
            nc.sync.dma_start(out=est, in_=t_lastyT.ap())
            c0 = const.tile([128, B], F32)
            nc.vector.memset(c0, 0.0)
            c1 = const.tile([128, B], F32)
            nc.vector.memset(c1, 0.0)

            def zpair():
                """Two PSUM banks, 2 gates each: A=(g,i), B=(f,o)."""
                zA = psum.tile([128, 2, B], F32, tag="z", name="zA")
                zB = psum.tile([128, 2, B], F32, tag="z", name="zB")
                return zA, zB

            def gates_mm(z, w, k, h, start, stop):
                """One k-tile of all 4 gates into the packed banks.

                start=True clears the whole bank's has_written bits, so emit
                it only on the first MM into each bank (slot 0); the slot-1
                opener relies on cleared has_written = overwrite semantics.
                One stop per bank likewise (on the last MM, slot 1)."""
                for bank, slot, g in PACK:
                    nc.tensor.matmul(z[bank][:, slot, :], w[:, k, g, :], h[:, k, :],
                                     start=start and slot == 0,
                                     stop=stop and slot == 1)

            def ag_launch(hk, tag):
                """Bounce the local [128,B] fp16 slice to DRAM, trigger AG.
                The bounce rides the Activation HWDGE queue (right after the
                halfstep ACTs that produced hk) so the Sync queue holds only
                gather triggers and fires them the moment a mesh completes."""
                inb = dbounce.tile([128, B], FR, tag="agi" + tag, name="agi" + tag)
                nc.scalar.dma_start(out=inb, in_=hk[:])
                outb = dshared.tile([NCORES * 128, B], FR, tag="ago" + tag,
                                    name="ago" + tag, addr_space="Shared")
                nc.gpsimd.collective_compute(
                    "AllGather", mybir.AluOpType.bypass, replica_groups=RG,
                    ins=[inb[:].opt()], outs=[outb[:].opt()],
                )
                return outb

            def ag_gather(outb, tag, plan):
                """DMA the gathered [8*128,B] DRAM buffer into SBUF
                [128, KT_H, B].  plan: list of (ktile_lo, ktile_hi, engine)
                chunks so k-outer consumers can chase the arrivals."""
                full = hfull.tile([128, KT_H, B], FR, tag="hf" + tag, name="hf" + tag)
                src = outb[:].rearrange("(k p) b -> p k b", p=128)
                for lo, hi, eng in plan:
                    eng.dma_start(out=full[:, lo:hi, :], in_=src[:, lo:hi, :])
                return full

            def lstm_halfstep(z, bias, cprev, tagp):
                """Gate activations + cell update from packed banks (zA, zB).
                Returns (c_new, h_new); h_new fp16 for the next matmuls."""
                zA, zB = z
                gg = gact.tile([128, B], F32, tag="gg", name="gg")
                gi = gact.tile([128, B], F32, tag="gi", name="gi")
                gf = gact.tile([128, B], F32, tag="gf", name="gf")
                go = gact.tile([128, B], F32, tag="go", name="go")
                nc.scalar.activation(gg, zA[:, 0, :], AF.Tanh, bias=bias[:, 2:3])
                nc.scalar.activation(gi, zA[:, 1, :], AF.Sigmoid, bias=bias[:, 0:1])
                nc.scalar.activation(gf, zB[:, 0, :], AF.Sigmoid, bias=bias[:, 1:2])
                nc.scalar.activation(go, zB[:, 1, :], AF.Sigmoid, bias=bias[:, 3:4])
                ig = gact.tile([128, B], F32, tag="ig", name="ig")
                nc.vector.tensor_mul(ig, gi, gg)
                fc_ = gact.tile([128, B], F32, tag="fc_", name="fc_")
                nc.vector.tensor_mul(fc_, gf, cprev)
                cnew = state.tile([128, B], F32, tag=tagp, name="cnew")
                nc.vector.tensor_add(cnew, fc_, ig)
                tc_ = gact.tile([128, B], F32, tag="tc_", name="tc_")
                nc.scalar.activation(tc_, cnew, AF.Tanh)
                hnew = state.tile([128, B], FR, tag=tagp + "h", name="hnew")
                nc.vector.tensor_mul(hnew, go, tc_)
                return cnew, hnew

            def load_xpre(s):
                # encode: gathers own DQ1 (Sync), so xt rides the Activation
                # queue where DQ10 is idle at body top
                xt = xload.tile([128, 2, B], FR, tag="x", name="xt")
                nc.scalar.dma_start(out=xt, in_=t_xpre.ap()[s])
                return xt

            def load_xfwd(t):
                # decode: the AG bounce-outs own DQ10 (Activation) mid-body,
                # so xt rides Sync/DQ1 which is idle there
                xt = xload.tile([128, 2, B], FR, tag="x", name="xt")
                nc.sync.dma_start(out=xt[:, 0, :], in_=t_xfwd.ap()[t, 0:128, :])
                nc.sync.dma_start(out=xt[0:64, 1, :], in_=t_xfwd.ap()[t, 128:XD, :])
                return xt

            # ================= encode =================
            h0f = None  # gathered h0_{s-1}
            h1f = None  # gathered h1_{s-2}
            xt = load_xpre(0)
            for s in range(PRE_LEN):
                # z0_s x-part — fills the AG(h0_{s-1}) window
                z0 = zpair()
                for kx in range(2):
                    gates_mm(z0, w0xT, kx, xt, start=(kx == 0), stop=(s == 0 and kx == 1))
                xt = load_xpre(s + 1) if s + 1 < PRE_LEN else None
                # z1_{s-1} whh1-part — window work too
                if s >= 1:
                    z1 = zpair()
                    if s >= 2:
                        for k in range(KT_H):
                            gates_mm(z1, whh1T, k, h1f, start=(k == 0), stop=False)
                # z0_s close (dep: h0f_{s-1}), k-outer chasing the gather
                if s >= 1:
                    for k in range(KT_H):
                        gates_mm(z0, whh0T, k, h0f, start=False, stop=(k == KT_H - 1))
                c0, h0k = lstm_halfstep(z0, b0, c0, "c0")
                ob0 = ag_launch(h0k, "0")
                # z1_{s-1} close — runs during AG(h0_s)
                if s >= 1:
                    for k in range(KT_H):
                        gates_mm(z1, wih1T, k, h0f, start=(s == 1 and k == 0),
                                 stop=(k == KT_H - 1))
                    c1, h1k = lstm_halfstep(z1, b1, c1, "c1")
                    ob1 = ag_launch(h1k, "1")
                # gathers on the Sync queue (a Scalar-queue chunk here would
                # stall the next halfstep's ACTs on an AG they don't need)
                h0f = ag_gather(ob0, "0", [(0, 4, nc.sync), (4, KT_H, nc.sync)])
                if s >= 1:
                    h1f = ag_gather(ob1, "1", [(0, 4, nc.sync), (4, KT_H, nc.sync)])

            # ---- encode flush: L1 of the last encode step (h1_63 = "top") ----
            z1 = zpair()
            for k in range(KT_H):
                gates_mm(z1, whh1T, k, h1f, start=(k == 0), stop=False)
            for k in range(KT_H):
                gates_mm(z1, wih1T, k, h0f, start=False, stop=(k == KT_H - 1))
            c1, h1k = lstm_halfstep(z1, b1, c1, "c1")
            ob1 = ag_launch(h1k, "1")
            h1f = ag_gather(ob1, "1", [(0, 4, nc.sync), (4, KT_H, nc.scalar)])

            # ================= decode =================
            xt = load_xfwd(0)
            for t in range(FWD_LEN):
                last = t == FWD_LEN - 1
                # z0_t partials (x + whh0@h0f_{t-1}) — run during AG(h1_{t-1})
                if not last:
                    z0 = zpair()
                    for bank, slot, g in PACK:
                        nc.tensor.matmul(z0[bank][:, slot, :], w0xT[:, 0, g, :],
                                         xt[:, 0, :], start=(slot == 0), stop=False)
                        nc.tensor.matmul(z0[bank][:, slot, :], w0xT[0:64, 1, g, :],
                                         xt[0:64, 1, :], start=False, stop=False)
                    for k in range(KT_H):
                        gates_mm(z0, whh0T, k, h0f, start=False, stop=False)
                    xt = load_xfwd(t + 1) if t + 1 < FWD_LEN - 1 else None

                # ---- FC head (replicated): est_t — dep h1f_{t-1} ----
                # k-outer: all 8 M-tiles accumulate concurrently in 4 packed
                # banks, so the matmuls chase the h1f gather chunks.
                ups = [psum.tile([128, 2, B], F32, tag="z", name=f"up{mb}")
                       for mb in range(KT_H // 2)]
                for k in range(KT_H):
                    for m in range(KT_H):
                        nc.tensor.matmul(ups[m // 2][:, m % 2, :],
                                         fcw1T[:, k, m * 128:(m + 1) * 128],
                                         h1f[:, k, :],
                                         start=(k == 0 and m % 2 == 0),
                                         stop=(k == KT_H - 1 and m % 2 == 1))
                pp = psum.tile([YD, B], F32, tag="z", name="pp")
                # fc2 bias opens the pp group (constant operands — fires the
                # moment the bank frees, off the ACT-gated tail)
                nc.tensor.matmul(pp, fcb2r[:, :], onesr[:, :], start=True, stop=False)
                us = []
                for m in range(KT_H):
                    u = gact.tile([128, B], FR, tag="u", name="u")
                    nc.scalar.activation(u, ups[m // 2][:, m % 2, :], AF.Tanh,
                                         bias=fcb1[:, m:m + 1])
                    us.append(u)
                    if m >= 1:
                        nc.tensor.matmul(pp, fcw2T[:, m - 1, :], us[m - 1],
                                         start=False, stop=False)
                nc.tensor.matmul(pp, fcw2T[:, KT_H - 1, :], us[KT_H - 1],
                                 start=False, stop=True)
                estn = state.tile([YD, B], F32, tag="est", name="estn")
                if not last:
                    # est_r feeds the critical west-close; emit it first
                    est_r = state.tile([YD, B], FR, tag="estr", name="est_r")
                    nc.vector.tensor_add(est_r, est, pp)
                nc.vector.tensor_add(estn, est, pp)
                nc.sync.dma_start(out=t_out.ap()[t], in_=estn)
                est = estn
                if last:
                    break
                # close z0_t with the est K-tile
                for bank, slot, g in PACK:
                    nc.tensor.matmul(z0[bank][:, slot, :], westT[:, g, :], est_r,
                                     start=False, stop=(slot == 1))
                c0, h0k = lstm_halfstep(z0, b0, c0, "c0")
                ob0 = ag_launch(h0k, "0")
                # z1_t whh1-part — runs during AG(h0_t)
                z1 = zpair()
                for k in range(KT_H):
                    gates_mm(z1, whh1T, k, h1f, start=(k == 0), stop=False)
                h0f = ag_gather(ob0, "0", [(0, 2, nc.sync), (2, 4, nc.scalar),
                                           (4, 6, nc.sync), (6, 8, nc.scalar)])
                # z1_t close — dep h0f_t, k-outer chasing the gather
                for k in range(KT_H):
                    gates_mm(z1, wih1T, k, h0f, start=False, stop=(k == KT_H - 1))
                c1, h1k = lstm_halfstep(z1, b1, c1, "c1")
                ob1 = ag_launch(h1k, "1")
                h1f = ag_gather(ob1, "1", [(0, 2, nc.sync), (2, 4, nc.scalar),
                                           (4, 6, nc.sync), (6, 8, nc.scalar)])

    nc.compile()
    return nc


def kernel(**inputs) -> np.ndarray:
    from concourse.bass_utils import run_bass_kernel_spmd

    key = "prog"
    if key not in _CACHE:
        _CACHE[key] = _build_program()
    nc = _CACHE[key]

    in_maps = _shard_host(inputs)
    res = run_bass_kernel_spmd(nc, in_maps, core_ids=list(range(NCORES)))
    est = np.asarray(res.results[0]["est_out"])  # (FWD, YD, B)
    return est.transpose(0, 2, 1).astype(np.float32).copy()  # (FWD, B, YD)
